# revision 1
# baseline (speedup 1.0000x reference)
"""Trainium2 Bass kernel for nn_ConversationAtt (sparse_attention).

Reference computation (per batch b, passage p):
    xx[p]   = x[b, :, p, :]                         # [Q=16, E=1024]
    rep     = relu(xx @ W^T)                        # [16, H=1024]
    score   = rep @ diag(D) @ rep^T                 # [16, 16]
    masked  = score + (-inf where key_mask & j>=i)
    prob    = softmax(masked, axis=-1)
    y[b, :, p, :] = prob @ xx

Sharding: data-parallel over batch B=8 -> one batch per NeuronCore.

Per-core schedule (P=512 passages = 16 chunks x 4 groups x 8 passages):
  - load x rows (p-major, q-minor) [128, 1024] f32 per group
  - PE-transpose to X^T [e, rows]; evac to SBUF
  - stage A (float32r, full PE rate): srepT[h, rows] = relu(Wst^T X^T),
    where Wst = (W * sqrt(D)[:,None])^T  (sqrt(D) folded into W since
    relu(z)*s == relu(z*s) for s>=0); stored bf16
  - stage B (bf16): score[128,128] = srepT_g^T srepT_g per group; only the
    8 diagonal 16x16 blocks are meaningful, the rest are masked to -inf
  - mask: additive tile built on GPSIMD from constants (affine_select) and
    the per-group key-mask row; softmax row-wise with fused exp+rowsum on
    ScalarE; normalization deferred to the y evacuation
  - prob^T via PE transpose; stage D (float32r): y = probT^T @ xx
  - y evac applies 1/rowsum; DMA out
"""

import numpy as np

import concourse.bass as bass
import concourse.tile as tile
from concourse import bacc, mybir
from concourse.bass_utils import run_bass_kernel_spmd
from concourse.masks import make_identity

F32 = mybir.dt.float32
F32R = mybir.dt.float32r
BF16 = mybir.dt.bfloat16
U8 = mybir.dt.uint8
AF = mybir.ActivationFunctionType
ALU = mybir.AluOpType

B, Q, P, E, H = 8, 16, 512, 1024, 1024
GP = 8            # passages per group
G = P // GP       # 64 groups
CH = 4            # groups per chunk
NCHUNK = G // CH  # 16 chunks
ROWS = GP * Q     # 128 rows per group
BIG = -1.0e30


def build_module(n_cores: int = 8, repeat: int = 1):
    nc = bacc.Bacc("TRN2", target_bir_lowering=False, debug=False,
                   num_devices=n_cores)
    x = nc.dram_tensor("x", [Q, P, E], F32R, kind="ExternalInput")
    mask = nc.dram_tensor("mask", [Q, P], U8, kind="ExternalInput")
    w = nc.dram_tensor("w", [H, E], F32, kind="ExternalInput")
    d = nc.dram_tensor("d", [H], F32, kind="ExternalInput")
    y = nc.dram_tensor("y", [Q, P, E], F32, kind="ExternalOutput")

    with tile.TileContext(nc) as tc:
        with (
            tc.tile_pool(name="const", bufs=1) as cpool,
            tc.tile_pool(name="wst", bufs=1) as wstpool,
            tc.tile_pool(name="xx", bufs=6) as xxpool,
            tc.tile_pool(name="xt", bufs=2) as xtpool,
            tc.tile_pool(name="srep", bufs=2) as srpool,
            tc.tile_pool(name="soft", bufs=4) as softpool,
            tc.tile_pool(name="ysb", bufs=3) as ypool,
            tc.tile_pool(name="tps", bufs=2, space="PSUM") as tpsp,
            tc.tile_pool(name="srps", bufs=2, space="PSUM") as srpsp,
            tc.tile_pool(name="scps", bufs=2, space="PSUM") as scpsp,
            tc.tile_pool(name="yps", bufs=2, space="PSUM") as ypsp,
        ):
            # ---------------- constants -------------------------------
            ident = cpool.tile([128, 128], F32, tag="ident")
            make_identity(nc, ident[:])
            identr = cpool.tile([128, 128], F32R, tag="identr")
            nc.scalar.copy(identr[:], ident[:])

            # TRI01[(p,i), (p',j)] = 1 where 16*p' + j - (16*p + i) >= 0
            # (covers p'==p & j>=i; also p'>p which C1BIG masks anyway)
            tri01 = cpool.tile([128, 128], F32, tag="tri01")
            nc.gpsimd.memset(tri01[:], 1.0)
            nc.gpsimd.affine_select(
                out=tri01[:], in_=tri01[:], compare_op=ALU.is_ge,
                fill=0.0, base=0, pattern=[[16, GP], [1, Q]],
                channel_multiplier=-1,
            )
            # C1BIG = BIG where p' != p, else 0
            c1big = cpool.tile([128, 128], F32, tag="c1big")
            nc.gpsimd.memset(c1big[:], 0.0)
            # keep where rho - 16*p' >= 0 (p >= p'), fill BIG where p' > p
            nc.gpsimd.affine_select(
                out=c1big[:], in_=c1big[:], compare_op=ALU.is_ge,
                fill=BIG, base=0, pattern=[[-16, GP], [0, Q]],
                channel_multiplier=1,
            )
            # keep where 16*p' - rho + 15 >= 0 (p' >= p), fill BIG where p' < p
            nc.gpsimd.affine_select(
                out=c1big[:], in_=c1big[:], compare_op=ALU.is_ge,
                fill=BIG, base=15, pattern=[[16, GP], [0, Q]],
                channel_multiplier=-1,
            )

            # ---------------- W * sqrt(D), transposed -----------------
            dcol = cpool.tile([128, 8], F32, tag="dcol")
            nc.sync.dma_start(dcol[:], d.rearrange("(t p) -> p t", p=128))
            sd = cpool.tile([128, 8], F32, tag="sd")
            nc.scalar.activation(sd[:], dcol[:], AF.Sqrt)

            wst = []  # wst[k]: [128 e, 1024 h]
            for k in range(8):
                wst.append(wstpool.tile([128, H], F32R, tag=f"wst{k}",
                                        name=f"wst{k}"))
            with tc.tile_pool(name="wnat", bufs=3) as wnpool:
                for t in range(8):
                    wn = wnpool.tile([128, E], F32, tag="wn")
                    nc.sync.dma_start(wn[:], w[t * 128:(t + 1) * 128, :])
                    nc.vector.tensor_scalar_mul(wn[:], wn[:], sd[:, t:t + 1])
                    for k in range(0, 8, 4):
                        wps = tpsp.tile([128, 512], F32, tag="tps")
                        for kk in range(4):
                            nc.tensor.transpose(
                                wps[:, kk * 128:(kk + 1) * 128],
                                wn[:, (k + kk) * 128:(k + kk + 1) * 128],
                                ident[:])
                        for kk in range(4):
                            nc.scalar.copy(
                                wst[k + kk][:, t * 128:(t + 1) * 128],
                                wps[:, kk * 128:(kk + 1) * 128])

            # ---------------- key-mask rows ---------------------------
            # KM[(p_local, j), g] = mask[j, 8g + p_local] * BIG
            kmu8 = cpool.tile([128, G], U8, tag="kmu8")
            # memset first: the rearranged-AP DMA confuses the sim's
            # per-byte init tracking (values verified correct regardless)
            nc.gpsimd.memset(kmu8[:], 0)
            nc.sync.dma_start(kmu8[:],
                              mask.rearrange("j (g p) -> p j g", p=GP))
            kmbig = cpool.tile([128, G], F32, tag="kmbig")
            nc.vector.tensor_scalar_mul(kmbig[:], kmu8[:], BIG)
            # one row per group, each on partition 0: via PE transpose
            # (kps[:64, :128] holds KM^T; row g = group g's mask row)
            kps = tpsp.tile([128, 512], F32, tag="tps")
            nc.tensor.transpose(kps[:64, :128], kmbig[:, :], ident[:])
            kmrall = cpool.tile([64, 128], F32, tag="kmrall")
            nc.vector.tensor_copy(kmrall[:], kps[:64, :128])
            # flatten to partition 0 (partition_broadcast needs base part 0)
            kmflat = cpool.tile([1, G * 128], F32, tag="kmflat")
            nc.sync.dma_start(kmflat[:], kmrall[:])

            # additive mask tiles for every group, built once upfront
            maskfull = []
            for g in range(G):
                bc = softpool.tile([128, 128], F32, tag="bc", name=f"bc{g}")
                nc.gpsimd.partition_broadcast(
                    bc[:], kmflat[0:1, g * 128:(g + 1) * 128])
                t1 = softpool.tile([128, 128], F32, tag="t1", name=f"t1{g}")
                nc.gpsimd.tensor_mul(t1[:], bc[:], tri01[:])
                mf = cpool.tile([128, 128], BF16, tag=f"mf{g}", name=f"mf{g}")
                nc.gpsimd.tensor_add(mf[:], t1[:], c1big[:])
                maskfull.append(mf)

            # ---------------- main loop -------------------------------
            for c in [ci for _ in range(repeat) for ci in range(NCHUNK)]:
                xxs = []
                for gl in range(CH):
                    g = c * CH + gl
                    xt_ = xxpool.tile([128, E], F32R, tag="xx")
                    nc.sync.dma_start(
                        xt_[:],
                        x[:, g * GP:(g + 1) * GP, :].rearrange(
                            "q p e -> p q e"))
                    xxs.append(xt_)

                # X^T for the chunk: xtk[k] = [128 e, 512 rows]
                xtk = []
                for k in range(8):
                    tp = tpsp.tile([128, 512], F32, tag="tps")
                    for gl in range(CH):
                        nc.tensor.transpose(
                            tp[:, gl * 128:(gl + 1) * 128].bitcast(F32R),
                            xxs[gl][:, k * 128:(k + 1) * 128], identr[:])
                    xt_ = xtpool.tile([128, 512], F32R, tag=f"xt{k}")
                    if k % 2 == 0:
                        nc.scalar.copy(xt_[:], tp[:])
                    else:
                        nc.vector.tensor_copy(xt_[:], tp[:])
                    xtk.append(xt_)

                # stage A: srepT[h] = relu(Wst^T @ X^T) -> bf16
                srt = []
                for h in range(8):
                    sp = srpsp.tile([128, 512], F32, tag="srps")
                    for k in range(8):
                        nc.tensor.matmul(
                            sp[:],
                            wst[k][:, h * 128:(h + 1) * 128],
                            xtk[k][:],
                            start=(k == 0), stop=(k == 7))
                    st = srpool.tile([128, 512], BF16, tag=f"sr{h}")
                    nc.scalar.activation(st[:], sp[:], AF.Relu)
                    srt.append(st)

                for gl in range(CH):
                    g = c * CH + gl
                    rsl = slice(gl * 128, (gl + 1) * 128)

                    # stage B: score (bf16), only diag 16x16 blocks valid
                    sc = scpsp.tile([128, 128], F32, tag="scps")
                    for h in range(8):
                        nc.tensor.matmul(sc[:], srt[h][:, rsl],
                                         srt[h][:, rsl],
                                         start=(h == 0), stop=(h == 7))

                    masked = softpool.tile([128, 128], F32, tag="msk")
                    nc.vector.tensor_add(masked[:], sc[:], maskfull[g][:])
                    negmax = softpool.tile([128, 1], F32, tag="ngm")
                    nc.vector.tensor_reduce(
                        negmax[:], masked[:], axis=mybir.AxisListType.X,
                        op=ALU.max, negate=True)
                    expd = softpool.tile([128, 128], F32, tag="exp")
                    sumexp = softpool.tile([128, 1], F32, tag="sum")
                    nc.scalar.activation(expd[:], masked[:], AF.Exp,
                                         bias=negmax[:], accum_out=sumexp[:])
                    recip = softpool.tile([128, 1], F32, tag="rcp")
                    nc.vector.reciprocal(recip[:], sumexp[:])

                    # probT via PE transpose
                    pt = tpsp.tile([128, 128], F32, tag="tps")
                    nc.tensor.transpose(pt[:], expd[:], ident[:])
                    probt = softpool.tile([128, 128], F32R, tag="pbt")
                    nc.vector.tensor_copy(probt[:], pt[:])

                    # stage D: y = probT^T @ xx  (f32r), normalize in evac
                    ysb = ypool.tile([128, E], F32, tag="y")
                    for half in range(2):
                        yp = ypsp.tile([128, 512], F32, tag="yps")
                        nc.tensor.matmul(
                            yp[:], probt[:],
                            xxs[gl][:, half * 512:(half + 1) * 512],
                            start=True, stop=True)
                        dst = ysb[:, half * 512:(half + 1) * 512]
                        if half == 0:
                            nc.vector.tensor_scalar_mul(dst, yp[:], recip[:])
                        else:
                            nc.scalar.mul(dst, yp[:], recip[:])

                    nc.scalar.dma_start(
                        y[:, g * GP:(g + 1) * GP, :].rearrange(
                            "q p e -> p q e"),
                        ysb[:])

    nc.finalize()
    return nc


_module_cache = {}


def _get_module(n_cores: int = 8):
    if n_cores not in _module_cache:
        _module_cache[n_cores] = build_module(n_cores)
    return _module_cache[n_cores]


def kernel(x: np.ndarray, mask: np.ndarray, W: np.ndarray,
           D: np.ndarray) -> np.ndarray:
    """Full-input entry point: shards over batch across 8 NeuronCores."""
    assert x.shape == (B, Q, P, E)
    nc = _get_module(B)
    mask_u8 = np.ascontiguousarray(mask).view(np.uint8)
    w32 = np.ascontiguousarray(W, dtype=np.float32)
    d32 = np.ascontiguousarray(D, dtype=np.float32)
    in_maps = [
        {"x": np.ascontiguousarray(x[b], dtype=np.float32),
         "mask": mask_u8[b], "w": w32, "d": d32}
        for b in range(B)
    ]
    res = run_bass_kernel_spmd(nc, in_maps, core_ids=list(range(B)))
    out = np.stack([r["y"] for r in res.results], axis=0)  # [B, Q, P, E]
    return out.reshape(B * Q, P, E)



# revision 22
# speedup vs baseline: 1.2957x; 1.2957x over previous
"""Trainium2 Bass kernel for nn_ConversationAtt (sparse_attention).

Reference computation (per batch b, passage p):
    xx[p]   = x[b, :, p, :]                         # [Q=16, E=1024]
    rep     = relu(xx @ W^T)                        # [16, H=1024]
    score   = rep @ diag(D) @ rep^T                 # [16, 16]
    masked  = score + (-inf where key_mask & j>=i)
    prob    = softmax(masked, axis=-1)
    y[b, :, p, :] = prob @ xx

Sharding: data-parallel over batch B=8 -> one batch per NeuronCore.

Per-core schedule (P=512 passages = 16 chunks x 4 groups x 8 passages),
software-pipelined so the in-order PE queue never head-of-line blocks on
the cross-engine softmax chain:

  body(i) for chunk c = seq[i]:
    - DMA x rows for chunk c+1 (prefetched one chunk ahead)
    - stage A (bf16): srepT[h, rows] = relu(Wst^T X^T) for 8 h-tiles,
      Wst = (W * sqrt(D)[:,None])^T folded at setup; relu evac on ScalarE
    - PE-transpose X^T for chunk c+1 (fills the window while softmax
      inputs for chunk c drain); evac psum->bf16 alternating Scalar/Vector
    - stage B (bf16): score[128,128] per group; mask-add + rowmax (DVE),
      fused exp+rowsum (ScalarE), reciprocal (DVE) -- no PE in this block
    - tail: probT via PE transpose, y = probT^T @ xx (f32r), 1/rowsum
      folded into the psum evac, DMA out

  W prep (DMA + scale + PE transpose + evac) is interleaved into chunk 0's
  stage A h-loop: A(0,h) only needs W rows [128h:128h+128].
"""

import numpy as np

import concourse.bass as bass
import concourse.tile as tile
from concourse import bacc, mybir
from concourse.bass_utils import run_bass_kernel_spmd
from concourse.masks import make_identity

F32 = mybir.dt.float32
F32R = mybir.dt.float32r
BF16 = mybir.dt.bfloat16
U8 = mybir.dt.uint8
AF = mybir.ActivationFunctionType
ALU = mybir.AluOpType

B, Q, P, E, H = 8, 16, 512, 1024, 1024
GP = 8            # passages per group
G = P // GP       # 64 groups
CH = 4            # groups per chunk
NCHUNK = G // CH  # 16 chunks
ROWS = GP * Q     # 128 rows per group
BIG = -1.0e30


def build_module(n_cores: int = 8, repeat: int = 1):
    nc = bacc.Bacc("TRN2", target_bir_lowering=False, debug=False,
                   num_devices=n_cores)
    x = nc.dram_tensor("x", [Q, P, E], F32R, kind="ExternalInput")
    mask = nc.dram_tensor("mask", [Q, P], U8, kind="ExternalInput")
    w = nc.dram_tensor("w", [H, E], F32, kind="ExternalInput")
    d = nc.dram_tensor("d", [H], F32, kind="ExternalInput")
    y = nc.dram_tensor("y", [Q, P, E], F32, kind="ExternalOutput")

    seq = [ci for _ in range(repeat) for ci in range(NCHUNK)]

    with tile.TileContext(nc) as tc:
        with (
            tc.tile_pool(name="const", bufs=1) as cpool,
            tc.tile_pool(name="wst", bufs=1) as wstpool,
            tc.tile_pool(name="wn", bufs=8) as wnpool,
            tc.tile_pool(name="xx", bufs=3) as xxpool,
            tc.tile_pool(name="xt", bufs=2) as xtpool,
            tc.tile_pool(name="srep", bufs=2) as srpool,
            tc.tile_pool(name="soft", bufs=4) as softpool,
            tc.tile_pool(name="ysb", bufs=3) as ypool,
            tc.tile_pool(name="xtps", bufs=3, space="PSUM") as xtps,
            tc.tile_pool(name="srps", bufs=3, space="PSUM") as srpsp,
            tc.tile_pool(name="scps", bufs=2, space="PSUM") as scpsp,
        ):
            # ---------------- constants -------------------------------
            ident = cpool.tile([128, 128], F32, tag="ident")
            make_identity(nc, ident[:])
            identr = cpool.tile([128, 128], F32R, tag="identr")
            nc.scalar.copy(identr[:], ident[:])

            # TRI01[(p,i), (p',j)] = 1 where 16*p' + j - (16*p + i) >= 0
            tri01 = cpool.tile([128, 128], F32, tag="tri01")
            nc.gpsimd.memset(tri01[:], 1.0)
            nc.gpsimd.affine_select(
                out=tri01[:], in_=tri01[:], compare_op=ALU.is_ge,
                fill=0.0, base=0, pattern=[[16, GP], [1, Q]],
                channel_multiplier=-1,
            )
            # C1BIG = BIG where p' != p, else 0
            c1big = cpool.tile([128, 128], F32, tag="c1big")
            nc.gpsimd.memset(c1big[:], 0.0)
            nc.gpsimd.affine_select(
                out=c1big[:], in_=c1big[:], compare_op=ALU.is_ge,
                fill=BIG, base=0, pattern=[[-16, GP], [0, Q]],
                channel_multiplier=1,
            )
            nc.gpsimd.affine_select(
                out=c1big[:], in_=c1big[:], compare_op=ALU.is_ge,
                fill=BIG, base=15, pattern=[[16, GP], [0, Q]],
                channel_multiplier=-1,
            )

            # ---------------- sqrt(D) ---------------------------------
            dcol = cpool.tile([128, 8], F32, tag="dcol")
            nc.sync.dma_start(dcol[:], d.rearrange("(t p) -> p t", p=128))
            sd = cpool.tile([128, 8], F32, tag="sd")
            nc.scalar.activation(sd[:], dcol[:], AF.Sqrt)

            # ---------------- key-mask rows (emitted later) -----------
            # kmflat[0, 128g + 16p + j] = mask[j, 8g + p] * BIG
            # Fast path: contiguous mask DMA -> scale -> 4 PE transposes
            # -> one strided SBUF->SBUF gather (64B lines).
            kmflat = cpool.tile([1, G * 128], F32, tag="kmflat")

            def emit_mask_load():
                mrow = cpool.tile([16, P], U8, tag="mrow")
                nc.sync.dma_start(mrow[:], mask[:, :])
                mbigf = cpool.tile([16, P], F32, tag="mbigf")
                nc.vector.tensor_scalar_mul(mbigf[:], mrow[:], BIG)
                pskm = scpsp.tile([128, 128], F32, tag="sp")
                for k in range(4):
                    nc.tensor.transpose(
                        pskm[:, 16 * k:16 * (k + 1)],
                        mbigf[:, 128 * k:128 * (k + 1)], ident[:16, :16])
                t1km = cpool.tile([128, 64], F32, tag="t1km")
                nc.vector.tensor_copy(t1km[:], pskm[:, :64])
                # t1km[8*gg + p, 16*k + j] -> kmflat[0, 128*(16k+gg)+16p+j]
                for k in range(4):
                    nc.sync.dma_start(
                        kmflat[:, 2048 * k:2048 * (k + 1)].rearrange(
                            "o (gp j) -> o gp j", j=16),
                        t1km[:, 16 * k:16 * (k + 1)])

            # additive mask tiles for every group (GPSIMD churns through
            # these in FIFO order while the main loop runs)
            maskfull = [None] * G

            def emit_mask_tiles():
                for g in range(G):
                    bc = softpool.tile([128, 128], F32, tag="bc",
                                       name=f"bc{g}")
                    nc.gpsimd.partition_broadcast(
                        bc[:], kmflat[0:1, g * 128:(g + 1) * 128])
                    t1 = softpool.tile([128, 128], F32, tag="t1",
                                       name=f"t1{g}")
                    nc.gpsimd.tensor_mul(t1[:], bc[:], tri01[:])
                    mf = cpool.tile([128, 128], BF16, tag=f"mf{g}",
                                    name=f"mf{g}")
                    nc.gpsimd.tensor_add(mf[:], t1[:], c1big[:])
                    maskfull[g] = mf

            # ---------------- W * sqrt(D), transposed, bf16 -----------
            wst = []  # wst[k]: [128 e, 1024 h]
            for k in range(8):
                wst.append(wstpool.tile([128, H], BF16, tag=f"wst{k}",
                                        name=f"wst{k}"))

            wn_tiles = {}

            def load_w_tile(t):
                wn = wnpool.tile([128, E], F32, tag="wn", name=f"wn{t}")
                nc.sync.dma_start(wn[:], w[t * 128:(t + 1) * 128, :])
                nc.vector.tensor_scalar_mul(wn[:], wn[:], sd[:, t:t + 1])
                wn_tiles[t] = wn

            def emit_w_tile(t):
                """Prep wst[:, 128t:128t+128] from W rows [128t:128t+128]."""
                wn = wn_tiles.pop(t)
                for k in range(0, 8, 4):
                    wps = xtps.tile([128, 512], F32, tag="tps")
                    for kk in range(4):
                        nc.tensor.transpose(
                            wps[:, kk * 128:(kk + 1) * 128],
                            wn[:, (k + kk) * 128:(k + kk + 1) * 128],
                            ident[:])
                    for kk in range(4):
                        if kk % 2 == 0:
                            nc.scalar.copy(
                                wst[k + kk][:, t * 128:(t + 1) * 128],
                                wps[:, kk * 128:(kk + 1) * 128])
                        else:
                            nc.vector.tensor_copy(
                                wst[k + kk][:, t * 128:(t + 1) * 128],
                                wps[:, kk * 128:(kk + 1) * 128])

            # ---------------- pipelined helpers -----------------------
            xx_map = {}   # i -> [4 x tile [128, E] f32r]
            xtk_map = {}  # i -> [8 x tile [128, 512] bf16]

            def load_x(i):
                c = seq[i]
                tiles = []
                for gl in range(CH):
                    g = c * CH + gl
                    t = xxpool.tile([128, E], F32R, tag=f"xx{gl}")
                    nc.sync.dma_start(
                        t[:],
                        x[:, g * GP:(g + 1) * GP, :].rearrange(
                            "q p e -> p q e"))
                    tiles.append(t)
                xx_map[i] = tiles

            def emit_xt(i):
                """PE-transpose chunk seq[i]'s x to X^T, evac to bf16."""
                xts = []
                for k in range(8):
                    tp = xtps.tile([128, 512], F32, tag="tps")
                    for gl in range(CH):
                        nc.tensor.transpose(
                            tp[:, gl * 128:(gl + 1) * 128].bitcast(F32R),
                            xx_map[i][gl][:, k * 128:(k + 1) * 128],
                            identr[:])
                    dst = xtpool.tile([128, 512], BF16, tag=f"xt{k}")
                    if k % 2 == 0:
                        nc.vector.tensor_copy(dst[:], tp[:])
                    else:
                        nc.scalar.copy(dst[:], tp[:])
                    xts.append(dst)
                xtk_map[i] = xts

            # ---------------- prologue --------------------------------
            # DMA-queue order: x(0) first (XT(0) gate), mask row (fast),
            # then W tiles interleaved with x(1) groups so neither path
            # head-blocks
            load_x(0)
            emit_mask_load()
            x1_tiles = []
            if len(seq) > 1:
                c1 = seq[1]
                for t in range(8):
                    load_w_tile(t)
                    if t == 1:
                        emit_mask_tiles()
                    if t % 2 == 1:
                        gl = t // 2
                        g = c1 * CH + gl
                        xt_ = xxpool.tile([128, E], F32R, tag=f"xx{gl}")
                        nc.sync.dma_start(
                            xt_[:],
                            x[:, g * GP:(g + 1) * GP, :].rearrange(
                                "q p e -> p q e"))
                        x1_tiles.append(xt_)
                xx_map[1] = x1_tiles
            else:
                for t in range(8):
                    load_w_tile(t)
                emit_mask_tiles()
            emit_xt(0)

            # ---------------- main loop -------------------------------
            for i, c in enumerate(seq):
                xtk = xtk_map.pop(i)
                xxs = xx_map[i]

                # stage A: srepT[h] = relu(Wst^T @ X^T) -> bf16
                srt = []
                for h in range(8):
                    if i == 0:
                        emit_w_tile(h)
                    sp = srpsp.tile([128, 512], F32, tag="srps")
                    for k in range(8):
                        nc.tensor.matmul(
                            sp[:],
                            wst[k][:, h * 128:(h + 1) * 128],
                            xtk[k][:],
                            start=(k == 0), stop=(k == 7))
                    st = srpool.tile([128, 512], BF16, tag=f"sr{h}")
                    nc.scalar.activation(st[:], sp[:], AF.Relu)
                    srt.append(st)

                # X^T for chunk i+1: PE work that fills the window while
                # chunk i's softmax chain drains on DVE/ScalarE
                if i + 1 < len(seq):
                    emit_xt(i + 1)
                if i + 2 < len(seq):
                    load_x(i + 2)

                # stage B + softmax for all groups (no PE dependency)
                exps = []
                for gl in range(CH):
                    g = c * CH + gl
                    rsl = slice(gl * 128, (gl + 1) * 128)
                    sc = scpsp.tile([128, 128], F32, tag="sp")
                    for h in range(8):
                        nc.tensor.matmul(sc[:], srt[h][:, rsl],
                                         srt[h][:, rsl],
                                         start=(h == 0), stop=(h == 7))
                    masked = softpool.tile([128, 128], F32, tag="msk")
                    nc.vector.tensor_add(masked[:], sc[:], maskfull[g][:])
                    negmax = softpool.tile([128, 1], F32, tag="ngm")
                    nc.vector.tensor_reduce(
                        negmax[:], masked[:], axis=mybir.AxisListType.X,
                        op=ALU.max, negate=True)
                    expd = softpool.tile([128, 128], F32, tag="exp")
                    sumexp = softpool.tile([128, 1], F32, tag="sum")
                    nc.scalar.activation(expd[:], masked[:], AF.Exp,
                                         bias=negmax[:], accum_out=sumexp[:])
                    recip = softpool.tile([128, 1], F32, tag="rcp")
                    nc.vector.reciprocal(recip[:], sumexp[:])
                    exps.append((expd, recip))

                # tail: probT via PE transpose, y = probT^T @ xx, DMA out
                for gl in range(CH):
                    g = c * CH + gl
                    expd, recip = exps[gl]
                    pt = scpsp.tile([128, 128], F32, tag="sp")
                    nc.tensor.transpose(pt[:], expd[:], ident[:])
                    probt = softpool.tile([128, 128], F32R, tag="pbt")
                    nc.vector.tensor_copy(probt[:], pt[:])

                    ysb = ypool.tile([128, E], F32, tag="y")
                    for half in range(2):
                        yp = xtps.tile([128, 512], F32, tag="tps")
                        nc.tensor.matmul(
                            yp[:], probt[:],
                            xxs[gl][:, half * 512:(half + 1) * 512],
                            start=True, stop=True)
                        dst = ysb[:, half * 512:(half + 1) * 512]
                        if half == 0:
                            nc.vector.tensor_scalar_mul(dst, yp[:], recip[:])
                        else:
                            nc.scalar.mul(dst, yp[:], recip[:])

                    nc.scalar.dma_start(
                        y[:, g * GP:(g + 1) * GP, :].rearrange(
                            "q p e -> p q e"),
                        ysb[:])
                del xx_map[i]

    nc.finalize()
    return nc


_module_cache = {}


def _get_module(n_cores: int = 8):
    if n_cores not in _module_cache:
        _module_cache[n_cores] = build_module(n_cores)
    return _module_cache[n_cores]


def kernel(x: np.ndarray, mask: np.ndarray, W: np.ndarray,
           D: np.ndarray) -> np.ndarray:
    """Full-input entry point: shards over batch across 8 NeuronCores."""
    assert x.shape == (B, Q, P, E)
    nc = _get_module(B)
    mask_u8 = np.ascontiguousarray(mask).view(np.uint8)
    w32 = np.ascontiguousarray(W, dtype=np.float32)
    d32 = np.ascontiguousarray(D, dtype=np.float32)
    in_maps = [
        {"x": np.ascontiguousarray(x[b], dtype=np.float32),
         "mask": mask_u8[b], "w": w32, "d": d32}
        for b in range(B)
    ]
    res = run_bass_kernel_spmd(nc, in_maps, core_ids=list(range(B)))
    out = np.stack([r["y"] for r in res.results], axis=0)  # [B, Q, P, E]
    return out.reshape(B * Q, P, E)
